# revision 1
# baseline (speedup 1.0000x reference)
"""TRN2 Bass kernel for nn_Attention_74242804679195 (Bahdanau attention scorer).

out[b, s] = softmax_s( vt . tanh(enc[b,s,:] @ w1.T + dec[b,:] @ w2.T) )
Shapes: enc [32, 4096, 512], dec [32, 512], w1/w2 [512, 512], vt [512].

Sharding: data-parallel over batch across 8 NeuronCores (4 sequences per
core); w1/w2/vt replicated; softmax over seq stays core-local.

Per-core pipeline (all on device):
 - one-time: PE-transpose w1/w2 into [h, o] tiles (fp32r); dec_t = dec @ w2.T
   on PE; per-batch dec_t broadcast tiles (scaled 1/128).
 - per 512-row group: one 1 MB DMA (dual HWDGE rings, alternating).
 - per 128-row chunk: 4 PE transposes of enc subtiles into ONE psum bank
   (single accumulation group, disjoint writes) -> one bulk copy to SBUF
   (fp32r, alternating ACT/DVE) -> 4 fp32r matmuls (K=128, N=512) into PSUM
   + 1 bias matmul (ones128/128 @ dec_rep; K=1 matmuls crash NRT) ->
   ACT tanh (bf16) -> DVE mult by broadcast vt + row reduce -> logit column.
 - per batch: exp (no max subtraction: |logit| <= ||vt||_1 ~ 18, safe in
   fp32), row sums, cross-partition sum via tiny DRAM roundtrip DMAs,
   reciprocal, scale, PE transpose [128,32] -> contiguous DMA out.
"""

import sys

for _p in ("/opt/trn_rl_repo", "/root/.axon_site/_ro/trn_rl_repo"):
    if _p not in sys.path:
        sys.path.insert(0, _p)

import numpy as np

import concourse.bass as bass
import concourse.tile as tile
from concourse import bacc, mybir
from concourse.masks import make_identity

F32 = mybir.dt.float32
F32R = mybir.dt.float32r
BF16 = mybir.dt.bfloat16
AF = mybir.ActivationFunctionType

B, S, H = 32, 4096, 512
NC = 8
BL = B // NC
KT = H // 128
NCHUNK = S // 128
CPD = 4  # chunks per DMA (1 MB transfers)


def _build():
    nc = bacc.Bacc("TRN2", target_bir_lowering=False, debug=False,
                   num_devices=NC)
    enc = nc.dram_tensor("enc", [BL, S, H], F32, kind="ExternalInput")
    dec = nc.dram_tensor("dec", [BL, H], F32, kind="ExternalInput")
    w1 = nc.dram_tensor("w1", [H, H], F32, kind="ExternalInput")
    w2 = nc.dram_tensor("w2", [H, H], F32, kind="ExternalInput")
    vt = nc.dram_tensor("vt", [H], F32, kind="ExternalInput")
    out = nc.dram_tensor("out", [BL, S], F32, kind="ExternalOutput")

    with tile.TileContext(nc) as tc:
        _body(nc, tc, enc, dec, w1, w2, vt, out)
    nc.compile()
    return nc


def _body(nc, tc, enc, dec, w1, w2, vt, out):
    with (
        tc.tile_pool(name="const", bufs=1) as const,
        tc.tile_pool(name="stage", bufs=2) as stage,
        tc.tile_pool(name="enc_nat", bufs=3) as enc_pool,
        tc.tile_pool(name="encT", bufs=4) as encT_pool,
        tc.tile_pool(name="tanh", bufs=3) as tanh_pool,
        tc.tile_pool(name="logit", bufs=2) as logit_pool,
        tc.tile_pool(name="soft", bufs=2) as soft_pool,
        tc.tile_pool(name="ps_tp", bufs=3, space="PSUM") as ps_tp,
        tc.tile_pool(name="ps_y", bufs=3, space="PSUM") as ps_y,
        tc.tile_pool(name="ps_soft", bufs=2, space="PSUM") as ps_soft,
        tc.tile_pool(name="dram", bufs=1, space="DRAM") as dram_pool,
    ):
        ident = const.tile([128, 128], F32)
        make_identity(nc, ident)
        identB = const.tile([BL, BL], F32)
        make_identity(nc, identB)

        ones_f = const.tile([128, 128], F32)
        nc.vector.memset(ones_f, 1.0)
        ones128 = const.tile([128, 128], F32R)
        nc.vector.tensor_copy(out=ones128, in_=ones_f)
        ones_col = const.tile([128, 1], F32)
        nc.vector.memset(ones_col, 1.0)

        vt_f = const.tile([128, H], F32)
        vt_ap = vt.ap()
        vt_bcast = bass.AP(tensor=vt_ap.tensor, offset=vt_ap.offset,
                           ap=[[0, 128]] + list(vt_ap.ap))
        nc.sync.dma_start(out=vt_f, in_=vt_bcast)
        vt_b = const.tile([128, H], BF16)
        nc.vector.tensor_copy(out=vt_b, in_=vt_f)

        w1T = [const.tile([128, H], F32R, tag=f"w1T{k}", name=f"w1T{k}")
               for k in range(KT)]
        w2T = [const.tile([128, H], F32R, tag=f"w2T{k}", name=f"w2T{k}")
               for k in range(KT)]
        for wsrc, wdstT in ((w1, w1T), (w2, w2T)):
            for j in range(KT):
                wn = stage.tile([128, H], F32, tag="wstage", name="wn")
                nc.sync.dma_start(out=wn,
                                  in_=wsrc.ap()[j * 128:(j + 1) * 128, :])
                for k in range(KT):
                    pt = ps_tp.tile([128, 128], F32, tag="tp", name="ptw")
                    nc.tensor.transpose(pt, wn[:, k * 128:(k + 1) * 128],
                                        ident)
                    nc.scalar.copy(out=wdstT[k][:, j * 128:(j + 1) * 128],
                                   in_=pt)

        # dec_t[b, o] = dec[b] @ w2.T
        dec_sb = stage.tile([BL, H], F32, tag="dec", name="dec_sb")
        nc.sync.dma_start(out=dec_sb, in_=dec.ap()[:, :])
        decT = [stage.tile([128, BL], F32R, tag=f"decT{k}", name=f"decT{k}")
                for k in range(KT)]
        for k in range(KT):
            pt = ps_tp.tile([128, BL], F32, tag="tp", name="ptd")
            nc.tensor.transpose(pt, dec_sb[:, k * 128:(k + 1) * 128], identB)
            nc.vector.tensor_copy(out=decT[k], in_=pt)
        dec_t_ps = ps_soft.tile([BL, H], F32, tag="soft", name="dec_t_ps")
        for k in range(KT):
            nc.tensor.matmul(dec_t_ps, decT[k], w2T[k],
                             start=(k == 0), stop=(k == KT - 1))
        dec_t_sb = stage.tile([BL, H], F32, tag="dec_t_sb", name="dec_t_sb")
        nc.vector.tensor_copy(out=dec_t_sb, in_=dec_t_ps)
        dec_dram = dram_pool.tile([BL, H], F32, name="dec_dram")
        nc.sync.dma_start(out=dec_dram, in_=dec_t_sb)
        dec_rep = []
        for b in range(BL):
            rowb = dec_dram[b, :]
            bcast = bass.AP(tensor=rowb.tensor, offset=rowb.offset,
                            ap=[[0, 128]] + list(rowb.ap))
            raw = const.tile([128, H], F32, tag=f"dec_raw{b}",
                             name=f"dec_raw{b}")
            nc.sync.dma_start(out=raw, in_=bcast)
            rep = const.tile([128, H], F32R, tag=f"dec_rep{b}",
                             name=f"dec_rep{b}")
            nc.scalar.mul(out=rep, in_=raw, mul=1.0 / 128)
            dec_rep.append(rep)

        for b in range(BL):
            logits = logit_pool.tile([128, NCHUNK], F32, tag="logits",
                                     name="logits")
            for g in range(NCHUNK // CPD):
                enc_grp = enc_pool.tile([128, CPD, H], F32, tag="enc",
                                        name="enc_grp")
                dma_eng = nc.scalar if g % 2 else nc.sync
                dma_eng.dma_start(
                    out=enc_grp,
                    in_=enc.ap()[b, g * CPD * 128:(g + 1) * CPD * 128, :]
                    .rearrange("(c p) h -> p c h", p=128))
                for ci in range(CPD):
                    c = g * CPD + ci
                    enc_nat = enc_grp[:, ci, :]
                    tp_bank = ps_tp.tile([128, KT, 128], F32, tag="tp",
                                         name="tp_bank")
                    for k in range(KT):
                        nc.tensor.matmul(
                            tp_bank[:, k, :],
                            enc_nat[:, k * 128:(k + 1) * 128], ident,
                            is_transpose=True,
                            start=(k == 0), stop=(k == KT - 1))
                    encT_all = encT_pool.tile([128, KT, 128], F32R,
                                              tag="encT", name="encT_all")
                    if c % 2 == 0:
                        nc.scalar.copy(out=encT_all, in_=tp_bank)
                    else:
                        nc.vector.tensor_copy(out=encT_all, in_=tp_bank)
                    y_ps = ps_y.tile([128, H], F32, tag="y", name="y_ps")
                    for k in range(KT):
                        nc.tensor.matmul(y_ps, encT_all[:, k, :], w1T[k],
                                         start=(k == 0), stop=False)
                    nc.tensor.matmul(y_ps, ones128, dec_rep[b],
                                     start=False, stop=True)
                    t_sb = tanh_pool.tile([128, H], BF16, tag="t", name="t_sb")
                    nc.scalar.activation(out=t_sb, in_=y_ps, func=AF.Tanh)
                    prod = tanh_pool.tile([128, H], BF16, tag="prod",
                                          name="prod")
                    nc.vector.tensor_mul(out=prod, in0=t_sb, in1=vt_b)
                    nc.vector.reduce_sum(out=logits[:, c:c + 1], in_=prod,
                                         axis=mybir.AxisListType.X)

            p_tile = soft_pool.tile([128, NCHUNK], F32, tag="p", name="p_tile")
            nc.scalar.activation(out=p_tile, in_=logits, func=AF.Exp)
            s1 = soft_pool.tile([128, 1], F32, tag="s1", name="s1")
            nc.vector.reduce_sum(out=s1, in_=p_tile,
                                 axis=mybir.AxisListType.X)
            s_dram = dram_pool.tile([128], F32, name=f"sd{b}")
            nc.sync.dma_start(out=s_dram, in_=s1)
            s_row = soft_pool.tile([1, 128], F32, tag="srow", name="s_row")
            nc.sync.dma_start(out=s_row, in_=s_dram.unsqueeze(0))
            tot = soft_pool.tile([1, 1], F32, tag="tot", name="tot")
            nc.vector.reduce_sum(out=tot, in_=s_row,
                                 axis=mybir.AxisListType.X)
            rec = soft_pool.tile([1, 1], F32, tag="rec", name="rec")
            nc.vector.reciprocal(out=rec, in_=tot)
            r_dram = dram_pool.tile([1], F32, name=f"rd{b}")
            nc.sync.dma_start(out=r_dram, in_=rec)
            rec_bc = soft_pool.tile([128, 1], F32, tag="recbc", name="rec_bc")
            rap = r_dram.unsqueeze(0)
            nc.sync.dma_start(
                out=rec_bc,
                in_=bass.AP(tensor=rap.tensor, offset=rap.offset,
                            ap=[[0, 128], [1, 1]]))
            nc.vector.tensor_scalar_mul(out=p_tile, in0=p_tile,
                                        scalar1=rec_bc)
            ot_ps = ps_soft.tile([NCHUNK, 128], F32, tag="soft", name="ot_ps")
            nc.tensor.transpose(ot_ps, p_tile, ident)
            o_sb = soft_pool.tile([NCHUNK, 128], F32, tag="osb", name="o_sb")
            nc.vector.tensor_copy(out=o_sb, in_=ot_ps)
            nc.sync.dma_start(
                out=out.ap()[b].rearrange("(c p) -> c p", p=128),
                in_=o_sb)


_NC_CACHE = None


def kernel(**inputs) -> np.ndarray:
    global _NC_CACHE
    from concourse.bass_utils import run_bass_kernel_spmd

    enc = np.ascontiguousarray(np.asarray(inputs["encoder_outputs"],
                                          dtype=np.float32))
    dec = np.ascontiguousarray(np.asarray(inputs["decoder_state"],
                                          dtype=np.float32))
    w1 = np.ascontiguousarray(np.asarray(inputs["w1"], dtype=np.float32))
    w2 = np.ascontiguousarray(np.asarray(inputs["w2"], dtype=np.float32))
    vt = np.ascontiguousarray(np.asarray(inputs["vt"], dtype=np.float32))

    if _NC_CACHE is None:
        _NC_CACHE = _build()
    nc = _NC_CACHE

    in_maps = []
    for c in range(NC):
        sl = slice(c * BL, (c + 1) * BL)
        in_maps.append({"enc": enc[sl], "dec": dec[sl],
                        "w1": w1, "w2": w2, "vt": vt})
    res = run_bass_kernel_spmd(nc, in_maps, list(range(NC)))
    return np.concatenate([res.results[c]["out"] for c in range(NC)],
                          axis=0).astype(np.float32)

